# revision 35
# baseline (speedup 1.0000x reference)
"""BPR-loss Trainium2 kernel.

Loss (see reference): for each graph b with tokens (logits lg, labels lb in
0..3) the per-graph log-prob is the mean over valid soft-labels s in {1,2,3}
of mean_{p: lb=s, n: lb<s} logsigmoid(lg_p - lg_n); loss = -mean over valid
graphs.

Everything after the pairwise logsigmoid is linear, so we:
  1. (host) sort each graph's tokens by label -> "neg" candidates for level s
     become a prefix [0, P_s) and "pos" candidates a contiguous range.
  2. (device) per graph-slot build D[n, p] = lg_p - lg_n with DVE/GPSIMD
     tensor_scalar (pos row broadcast via stride-0 DMA, neg column as the
     per-partition scalar), apply one wide ACT pass (softplus(-D) =
     -logsigmoid(D)), contract over n with the 0/1 prefix-mask matrix
     B[n, 3] on PE (M=3 -> LDWEIGHTS is ~free), accumulate into
     column-packed PSUM tiles, DMA raw PSUM out.
  3. (host) weight by exact pos-masks/scales, reduce to the scalar.

8 NeuronCores, data-parallel over graphs: graphs are sorted by work and
dealt in groups of 8 (one per core) so the SPMD program (shapes = group max)
is identical across cores and inherently load-balanced.
"""

import os
import sys

import numpy as np

for _p in ("/opt/trn_rl_repo", "/root/.axon_site/_ro/trn_rl_repo"):
    if os.path.isdir(_p) and _p not in sys.path:
        sys.path.append(_p)

NCORES = 8
MAXLEN = 256
NLAB = 4  # soft-label count (labels 0..3)
W_SUPER = 1536  # super-tile width (free elems) per ACT instruction
PSUM_W = 512  # psum bank width in f32
ALIGN = 2
PSUM_BANKS_PER_GROUP = 4  # psum tensor = 4 banks -> one wide staging copy
# "softplus": single ACT pass (HW table). "sigmoid_ln": two passes, exact-ish.
ACT_MODE = os.environ.get("BPR_ACT_MODE", "softplus")
N_DMA_SPLIT = 1  # bcast DMAs per super-tile (>1 overflows ISA sync-wait slots)


def _plan(logits, labels, s_num):
    """Host prep: per-graph label-sort + slotting + packing. Pure numpy."""
    B = int(s_num.shape[0])
    T = int(logits.shape[0])
    s_num = s_num.astype(np.int64)
    ends = np.cumsum(s_num)
    offs = ends - s_num

    graphs = []
    for b in range(B):
        s_eff = int(min(s_num[b], MAXLEN))
        lo = int(min(offs[b], T))
        hi = int(min(lo + s_eff, T))
        lg = logits[lo:hi].astype(np.float32)
        lb = labels[lo:hi].astype(np.int64)
        s_eff = lg.shape[0]
        order = np.argsort(lb, kind="stable")
        lgs = lg[order]
        c = np.bincount(lb, minlength=NLAB)[:NLAB]
        P = np.cumsum(c)  # P[s-1] = #{lb < s}
        P1, P2, P3 = int(P[0]), int(P[1]), int(P[2])
        c0 = int(c[0])
        Cp = s_eff - c0  # pos-candidate count (labels >= 1)
        valid = np.array(
            [(c[s] > 0) and (P[s - 1] > 0) for s in (1, 2, 3)], dtype=bool
        )
        cnt = int(valid.sum())
        gvalid = (int(s_num[b]) > 1) and (cnt > 0)
        # first pos-col whose label's neg-prefix extends past row 128 (the
        # only columns the second n-chunk can contribute to)
        start2 = None
        if P3 > 128:
            for s in (1, 2, 3):
                if P[s - 1] > 128:
                    start2 = sum(int(c[s_]) for s_ in range(1, s))
                    break
        # A[s-1, j]: weight of OUT[s-1, j] (j indexes pos cols = sorted labels
        # 1..3). Nonzero only on the row matching the col's label.
        A = np.zeros((3, max(Cp, 1)), dtype=np.float64)
        if gvalid:
            for s in (1, 2, 3):
                if valid[s - 1]:
                    j0 = int(P[s - 1]) - c0
                    j1 = j0 + int(c[s])
                    A[s - 1, j0:j1] = 1.0 / (float(c[s]) * float(P[s - 1]) * cnt)
        graphs.append(
            dict(
                b=b,
                s_eff=s_eff,
                c0=c0,
                P3=P3,
                Cp=Cp,
                lgs=lgs,
                P=(P1, P2, P3),
                A=A,
                gvalid=gvalid,
                start2=start2,
            )
        )

    n_valid = max(sum(g["gvalid"] for g in graphs), 1)

    # --- slotting: sort by work key desc, deal groups of NCORES ---
    nslots = (B + NCORES - 1) // NCORES
    work = np.array(
        [
            (0 if (g["P3"] == 0 or g["Cp"] == 0) else np.ceil(g["P3"] / 128) * g["Cp"])
            for g in graphs
        ]
    )
    order = np.argsort(-work, kind="stable")
    slots = []  # per slot: members (graph idx per core, may be None), shapes
    for k in range(nslots):
        members = [None] * NCORES
        p3m, cpm = 0, 0
        x2 = None
        for c_ in range(NCORES):
            i = k * NCORES + c_
            if i < B:
                g = graphs[int(order[i])]
                members[c_] = int(order[i])
                if g["P3"] > 0 and g["Cp"] > 0:
                    p3m = max(p3m, g["P3"])
                    cpm = max(cpm, g["Cp"])
                    if g["start2"] is not None:
                        x2 = g["start2"] if x2 is None else min(x2, g["start2"])
        cpm = -(-cpm // ALIGN) * ALIGN
        if p3m == 0 or cpm == 0:
            continue
        if p3m > 128 and x2 is None:
            x2 = 0
        if x2 is not None:
            x2 = (x2 // ALIGN) * ALIGN
        slots.append(dict(members=members, P3=p3m, Cp=cpm, x2=x2))

    # --- stripes: (slot, n-chunk). Consecutive per slot. The second n-chunk
    # only covers the pos-column suffix [x2, Cp) it can contribute to. ---
    stripes = []
    for si, sl in enumerate(slots):
        nch = -(-sl["P3"] // 128)
        for j in range(nch):
            K = min(128, sl["P3"] - 128 * j)
            x0 = 0 if j == 0 else sl["x2"]
            stripes.append(
                dict(slot=si, chunk=j, nch=nch, K=K, W=sl["Cp"] - x0, x0=x0)
            )
    nstripes = len(stripes)

    # --- super-tile packing (ACT granularity) ---
    supertiles = []
    cur = None
    goff = 0
    for j, st in enumerate(stripes):
        if cur is None or cur["W"] + st["W"] > W_SUPER:
            cur = dict(W=0, g0=goff, stripes=[])
            supertiles.append(cur)
        st["t"] = len(supertiles) - 1
        st["soff"] = cur["W"]
        st["goff"] = goff
        cur["stripes"].append(j)
        cur["W"] += st["W"]
        goff += st["W"]
    Wtot = goff

    # --- psum col packing: 512-col banks, grouped 4 banks per psum tensor ---
    ptiles = []
    pcur = None
    for si, sl in enumerate(slots):
        if pcur is None or pcur["W"] + sl["Cp"] > PSUM_W:
            pcur = dict(W=0, slots=[])
            ptiles.append(pcur)
        sl["pt"] = len(ptiles) - 1
        sl["coff"] = pcur["W"]
        pcur["slots"].append(si)
        pcur["W"] += sl["Cp"]
    ntiles = len(ptiles)
    ngroups = -(-ntiles // PSUM_BANKS_PER_GROUP)

    # --- engine assignment for the subtract (balance DVE vs GPSIMD, ns).
    # One engine per super-tile (keeps ACT waits at <= 2 sync slots).
    # DVE also owns the psum->sbuf staging copies; bias its budget. ---
    tdve = ngroups * (120.0 + PSUM_BANKS_PER_GROUP * PSUM_W / 2.0) / 0.96
    tgp = 0.0
    for sup in supertiles:
        sts = [stripes[j] for j in sup["stripes"]]
        cd = sum((58.0 + st["W"] / 2.0) / 0.96 for st in sts)
        cg = sum((150.0 + st["W"] * 1.03) / 1.2 for st in sts)
        if tdve + cd <= tgp + cg:
            sup["eng"] = 0
            tdve += cd
        else:
            sup["eng"] = 1
            tgp += cg
        for st in sts:
            st["eng"] = sup["eng"]

    # --- per-core input arrays ---
    posrow = np.zeros((NCORES, max(Wtot, 1)), dtype=np.float32)
    negcol = np.zeros((NCORES, 128, max(nstripes, 1)), dtype=np.float32)
    bmask = np.zeros((NCORES, 128, max(3 * nstripes, 1)), dtype=np.float32)
    for j, st in enumerate(stripes):
        sl = slots[st["slot"]]
        for c_ in range(NCORES):
            gi = sl["members"][c_]
            if gi is None:
                continue
            g = graphs[gi]
            if g["P3"] == 0 or g["Cp"] == 0:
                continue
            lgs = g["lgs"]
            c0 = g["c0"]
            x0 = st["x0"]
            if g["Cp"] > x0:
                posrow[c_, st["goff"] : st["goff"] + g["Cp"] - x0] = lgs[
                    c0 + x0 :
                ]
            n0 = 128 * st["chunk"]
            n1 = min(g["P3"], n0 + 128)
            if n1 > n0:
                negcol[c_, 0 : n1 - n0, j] = lgs[n0:n1]
                for s in (1, 2, 3):
                    Ps = g["P"][s - 1]
                    r1 = min(Ps, n1) - n0
                    if r1 > 0:
                        bmask[c_, 0:r1, 3 * j + (s - 1)] = 1.0

    return dict(
        graphs=graphs,
        slots=slots,
        stripes=stripes,
        supertiles=supertiles,
        ptiles=ptiles,
        nstripes=nstripes,
        ntiles=ntiles,
        ngroups=ngroups,
        Wtot=Wtot,
        n_valid=n_valid,
        posrow=posrow,
        negcol=negcol,
        bmask=bmask,
    )


def _signature(plan):
    """Program-shape signature for caching the compiled module."""
    sig = [plan["Wtot"], plan["nstripes"], plan["ntiles"], ACT_MODE]
    for st in plan["stripes"]:
        sig += [st["slot"], st["chunk"], st["K"], st["W"], st["t"], st["soff"], st["eng"]]
    for sl in plan["slots"]:
        sig += [sl["pt"], sl["coff"], sl["Cp"], sl["P3"]]
    return tuple(sig)


def _out_loc(sl):
    """(row0, col0) of a slot's [3, Cp] stripe in the dram out tensor."""
    pt = sl["pt"]
    g = pt // PSUM_BANKS_PER_GROUP
    col = (pt % PSUM_BANKS_PER_GROUP) * PSUM_W + sl["coff"]
    return 3 * g, col


def _emulate(plan):
    """Numpy emulation of the device program (for correctness of packing)."""
    GRPW = PSUM_BANKS_PER_GROUP * PSUM_W
    outs = []
    for c_ in range(NCORES):
        out = np.zeros((plan["ngroups"] * 3, GRPW), dtype=np.float32)
        for j, st in enumerate(plan["stripes"]):
            sl = plan["slots"][st["slot"]]
            w = st["W"]
            pos = plan["posrow"][c_][st["goff"] : st["goff"] + w]  # [w]
            neg = plan["negcol"][c_][0 : st["K"], j]  # [K]
            d = pos[None, :] - neg[:, None]  # [K, w] = lg_p - lg_n
            if ACT_MODE == "softplus":
                val = np.logaddexp(0.0, -d)  # softplus(-d)
            else:
                val = np.log(1.0 / (1.0 + np.exp(-d)))  # logsigmoid(d)
            bm = plan["bmask"][c_][0 : st["K"], 3 * j : 3 * j + 3]  # [K, 3]
            acc = bm.T.astype(np.float32) @ val.astype(np.float32)  # [3, w]
            r0, c0 = _out_loc(sl)
            c0 += st["x0"]
            out[r0 : r0 + 3, c0 : c0 + w] += acc
        outs.append(out)
    return outs


def _epilogue(plan, outs):
    total = 0.0
    for c_ in range(NCORES):
        out = outs[c_]
        for si, sl in enumerate(plan["slots"]):
            gi = sl["members"][c_]
            if gi is None:
                continue
            g = plan["graphs"][gi]
            if g["P3"] == 0 or g["Cp"] == 0 or not g["gvalid"]:
                continue
            r0, c0 = _out_loc(sl)
            O = out[r0 : r0 + 3, c0 : c0 + g["Cp"]].astype(np.float64)
            total += float((g["A"][:, : g["Cp"]] * O).sum())
    if ACT_MODE == "softplus":
        loss = total / plan["n_valid"]
    else:
        loss = -total / plan["n_valid"]
    return np.float32(loss)


_PROG_CACHE = {}


def _build_program(plan):
    import concourse.bass as bass  # noqa: F401
    import concourse.tile as tile
    from concourse import bacc, mybir
    from contextlib import ExitStack

    f32 = mybir.dt.float32
    nc = bacc.Bacc("TRN2", target_bir_lowering=False, debug=False,
                   num_devices=NCORES)
    GRPW = PSUM_BANKS_PER_GROUP * PSUM_W
    posrow = nc.dram_tensor("posrow", [1, max(plan["Wtot"], 1)], f32,
                            kind="ExternalInput")
    negcol = nc.dram_tensor("negcol", [128, max(plan["nstripes"], 1)], f32,
                            kind="ExternalInput")
    bmask = nc.dram_tensor("bmask", [128, max(3 * plan["nstripes"], 1)], f32,
                           kind="ExternalInput")
    out = nc.dram_tensor("out", [max(plan["ngroups"], 1) * 3, GRPW], f32,
                         kind="ExternalOutput")

    EXP = mybir.ActivationFunctionType.Exp
    LN = mybir.ActivationFunctionType.Ln

    stripes, slots = plan["stripes"], plan["slots"]
    # per psum group: index of last stripe writing it (for staging placement)
    last_stripe_of_group = {}
    for j, st in enumerate(stripes):
        grp = slots[st["slot"]]["pt"] // PSUM_BANKS_PER_GROUP
        last_stripe_of_group[grp] = j

    with tile.TileContext(nc) as tc, ExitStack() as ctx:
        cpool = ctx.enter_context(tc.tile_pool(name="const", bufs=1))
        dpool = ctx.enter_context(tc.tile_pool(name="d", bufs=3))
        lpool = ctx.enter_context(tc.tile_pool(name="ls", bufs=3))
        l2pool = ctx.enter_context(tc.tile_pool(name="ls2", bufs=3))
        ppool = ctx.enter_context(tc.tile_pool(name="ps", bufs=2, space="PSUM"))
        spool = ctx.enter_context(tc.tile_pool(name="stage", bufs=2))

        negt = cpool.tile([128, max(plan["nstripes"], 1)], f32, tag="negt")
        nc.sync.dma_start(out=negt[:, :], in_=negcol.ap()[:, :])
        bt = cpool.tile([128, max(3 * plan["nstripes"], 1)], f32, tag="bt")
        nc.sync.dma_start(out=bt[:, :], in_=bmask.ap()[:, :])
        # zero stationary + dummy row: K=1 matmuls that zero psum gap columns
        zstat = cpool.tile([1, 3], f32, tag="zstat")
        nc.vector.memset(zstat[:, :], 0.0)
        zrow = cpool.tile([1, PSUM_W], f32, tag="zrow")
        nc.vector.memset(zrow[:, :], 0.0)
        # joiners: 1-element reads that absorb DMA waits so TensorScalar
        # instructions never need more than one sync-wait slot
        joinv = cpool.tile([1, 2], f32, tag="joinv")
        joing = cpool.tile([1, 2], f32, tag="joing")
        nc.vector.tensor_copy(joinv[0:1, 0:1], negt[0:1, 0:1])
        nc.gpsimd.tensor_copy(joing[0:1, 0:1], negt[0:1, 0:1])

        def zero_gaps(pt, grp):
            ptl = plan["ptiles"]
            for b in range(PSUM_BANKS_PER_GROUP):
                pi = grp * PSUM_BANKS_PER_GROUP + b
                used = ptl[pi]["W"] if pi < len(ptl) else 0
                if used < PSUM_W:
                    gw = PSUM_W - used
                    nc.tensor.matmul(
                        out=pt[0:3, b * PSUM_W + used : (b + 1) * PSUM_W],
                        lhsT=zstat[0:1, 0:3],
                        rhs=zrow[0:1, 0:gw],
                        start=True,
                        stop=True,
                    )

        ptile_objs = {}

        for t, sup in enumerate(plan["supertiles"]):
            W = sup["W"]
            dt_ = dpool.tile([128, W], f32, tag="d")
            # broadcast pos rows across partitions via stride-0 DMA
            nsp = min(N_DMA_SPLIT, max(1, W // 256))
            bounds = [round(i * W / nsp) for i in range(nsp + 1)]
            for a, b_ in zip(bounds[:-1], bounds[1:]):
                if b_ > a:
                    src = posrow.ap()[0:1, sup["g0"] + a : sup["g0"] + b_]
                    nc.sync.dma_start(
                        out=dt_[:, a:b_], in_=src.partition_broadcast(128)
                    )
            # in-place subtract: d = lg_p - lg_n (single engine per super-tile)
            eng = nc.vector if sup["eng"] == 0 else nc.gpsimd
            ejoin = joinv if sup["eng"] == 0 else joing
            eng.tensor_copy(ejoin[0:1, 1:2], dt_[0:1, 0:1])
            for j in sup["stripes"]:
                st = stripes[j]
                K, w, so = st["K"], st["W"], st["soff"]
                eng.tensor_scalar_sub(
                    dt_[0:K, so : so + w],
                    dt_[0:K, so : so + w],
                    negt[0:K, j : j + 1],
                )
            # softplus(lg_n - lg_p) = ln(1 + exp(-(lg_p - lg_n))); Exp and Ln
            # share one ACT table set (natural_log_exp_and_others). Ln writes
            # a separate pool so exp's slot-WAR stays on the ACT engine.
            ls_ = lpool.tile([128, W], f32, tag="ls")
            nc.scalar.activation(ls_[:, :], dt_[:, :], EXP, bias=0.0, scale=-1.0)
            l2_ = l2pool.tile([128, W], f32, tag="ls2")
            nc.scalar.activation(l2_[:, :], ls_[:, :], LN, bias=1.0, scale=1.0)
            # contraction over n with prefix masks
            for j in sup["stripes"]:
                st = stripes[j]
                sl = slots[st["slot"]]
                K, w, so = st["K"], st["W"], st["soff"]
                grp = sl["pt"] // PSUM_BANKS_PER_GROUP
                if grp not in ptile_objs:
                    ptile_objs[grp] = ppool.tile(
                        [3, GRPW], f32, tag="ps", name=f"ps{grp}"
                    )
                    zero_gaps(ptile_objs[grp], grp)
                pt = ptile_objs[grp]
                c0_ = (
                    (sl["pt"] % PSUM_BANKS_PER_GROUP) * PSUM_W
                    + sl["coff"]
                    + st["x0"]
                )
                nc.tensor.matmul(
                    out=pt[0:3, c0_ : c0_ + w],
                    lhsT=bt[0:K, 3 * j : 3 * j + 3],
                    rhs=l2_[0:K, so : so + w],
                    start=(st["chunk"] == 0),
                    stop=(st["chunk"] == st["nch"] - 1),
                )
                if last_stripe_of_group.get(grp) == j:
                    stg = spool.tile([3, GRPW], f32, tag="stage",
                                     name=f"stg{grp}")
                    nc.vector.tensor_copy(joinv[0:1, 0:1], pt[0:1, 0:1])
                    nc.vector.tensor_copy(stg[:, :], pt[:, :])
                    nc.sync.dma_start(
                        out=out.ap()[3 * grp : 3 * grp + 3, :],
                        in_=stg[:, :],
                    )
                    del ptile_objs[grp]
    nc.compile()
    return nc


def _run_device(plan, trace=False):
    from concourse.bass_utils import run_bass_kernel_spmd

    sig = _signature(plan)
    if sig not in _PROG_CACHE:
        _PROG_CACHE[sig] = _build_program(plan)
    nc = _PROG_CACHE[sig]
    in_maps = [
        {
            "posrow": plan["posrow"][c_][None, :],
            "negcol": np.ascontiguousarray(plan["negcol"][c_]),
            "bmask": np.ascontiguousarray(plan["bmask"][c_]),
        }
        for c_ in range(NCORES)
    ]
    res = run_bass_kernel_spmd(
        nc, in_maps, core_ids=list(range(NCORES)), trace=trace
    )
    kernel._last_results = res
    return [res.results[c_]["out"] for c_ in range(NCORES)]


def kernel(logits, labels, s_num, _emulate_only=False, _trace=False):
    logits = np.asarray(logits)
    labels = np.asarray(labels)
    s_num = np.asarray(s_num)
    plan = _plan(logits, labels, s_num)
    if plan["nstripes"] == 0:
        return np.float32(0.0)
    if _emulate_only:
        outs = _emulate(plan)
    else:
        outs = _run_device(plan, trace=_trace)
    return _epilogue(plan, outs)


kernel._last_results = None


# revision 38
# speedup vs baseline: 2.0999x; 2.0999x over previous
"""BPR-loss Trainium2 kernel.

Loss (see reference): for each graph b with tokens (logits lg, labels lb in
0..3) the per-graph log-prob is the mean over valid soft-labels s in {1,2,3}
of mean_{p: lb=s, n: lb<s} logsigmoid(lg_p - lg_n); loss = -mean over valid
graphs.

Everything after the pairwise logsigmoid is linear, so we:
  1. (host) sort each graph's tokens by label -> "neg" candidates for level s
     become a prefix [0, P_s) and "pos" candidates a contiguous range.
  2. (device) per graph-slot build D[n, p] = lg_p - lg_n with DVE/GPSIMD
     tensor_scalar (pos row broadcast via stride-0 DMA, neg column as the
     per-partition scalar), apply one wide ACT pass (softplus(-D) =
     -logsigmoid(D)), contract over n with the 0/1 prefix-mask matrix
     B[n, 3] on PE (M=3 -> LDWEIGHTS is ~free), accumulate into
     column-packed PSUM tiles, DMA raw PSUM out.
  3. (host) weight by exact pos-masks/scales, reduce to the scalar.

8 NeuronCores, data-parallel over graphs: graphs are sorted by work and
dealt in groups of 8 (one per core) so the SPMD program (shapes = group max)
is identical across cores and inherently load-balanced.
"""

import os
import sys

import numpy as np

for _p in ("/opt/trn_rl_repo", "/root/.axon_site/_ro/trn_rl_repo"):
    if os.path.isdir(_p) and _p not in sys.path:
        sys.path.append(_p)

NCORES = 8
MAXLEN = 256
NLAB = 4  # soft-label count (labels 0..3)
W_SUPER = 1536  # super-tile width (free elems) per ACT instruction
PSUM_W = 512  # psum bank width in f32
ALIGN = 2
PSUM_BANKS_PER_GROUP = 4  # psum tensor = 4 banks -> one wide staging copy
# "softplus": single ACT pass (HW table). "sigmoid_ln": two passes, exact-ish.
ACT_MODE = os.environ.get("BPR_ACT_MODE", "softplus")
N_DMA_SPLIT = 1  # bcast DMAs per super-tile (>1 overflows ISA sync-wait slots)


def _plan(logits, labels, s_num):
    """Host prep: per-graph label-sort + slotting + packing. Pure numpy."""
    B = int(s_num.shape[0])
    T = int(logits.shape[0])
    s_num = s_num.astype(np.int64)
    ends = np.cumsum(s_num)
    offs = ends - s_num

    graphs = []
    for b in range(B):
        s_eff = int(min(s_num[b], MAXLEN))
        lo = int(min(offs[b], T))
        hi = int(min(lo + s_eff, T))
        lg = logits[lo:hi].astype(np.float32)
        lb = labels[lo:hi].astype(np.int64)
        s_eff = lg.shape[0]
        order = np.argsort(lb, kind="stable")
        lgs = lg[order]
        c = np.bincount(lb, minlength=NLAB)[:NLAB]
        P = np.cumsum(c)  # P[s-1] = #{lb < s}
        P1, P2, P3 = int(P[0]), int(P[1]), int(P[2])
        c0 = int(c[0])
        Cp = s_eff - c0  # pos-candidate count (labels >= 1)
        valid = np.array(
            [(c[s] > 0) and (P[s - 1] > 0) for s in (1, 2, 3)], dtype=bool
        )
        cnt = int(valid.sum())
        gvalid = (int(s_num[b]) > 1) and (cnt > 0)
        # first pos-col whose label's neg-prefix extends past row 128 (the
        # only columns the second n-chunk can contribute to)
        start2 = None
        if P3 > 128:
            for s in (1, 2, 3):
                if P[s - 1] > 128:
                    start2 = sum(int(c[s_]) for s_ in range(1, s))
                    break
        # A[s-1, j]: weight of OUT[s-1, j] (j indexes pos cols = sorted labels
        # 1..3). Nonzero only on the row matching the col's label.
        A = np.zeros((3, max(Cp, 1)), dtype=np.float64)
        if gvalid:
            for s in (1, 2, 3):
                if valid[s - 1]:
                    j0 = int(P[s - 1]) - c0
                    j1 = j0 + int(c[s])
                    A[s - 1, j0:j1] = 1.0 / (float(c[s]) * float(P[s - 1]) * cnt)
        graphs.append(
            dict(
                b=b,
                s_eff=s_eff,
                c0=c0,
                P3=P3,
                Cp=Cp,
                lgs=lgs,
                P=(P1, P2, P3),
                A=A,
                gvalid=gvalid,
                start2=start2,
            )
        )

    n_valid = max(sum(g["gvalid"] for g in graphs), 1)

    # --- slotting: sort by work key desc, deal groups of NCORES ---
    nslots = (B + NCORES - 1) // NCORES
    work = np.array(
        [
            (0 if (g["P3"] == 0 or g["Cp"] == 0) else np.ceil(g["P3"] / 128) * g["Cp"])
            for g in graphs
        ]
    )
    order = np.argsort(-work, kind="stable")
    slots = []  # per slot: members (graph idx per core, may be None), shapes
    for k in range(nslots):
        members = [None] * NCORES
        p3m, cpm = 0, 0
        x2 = None
        for c_ in range(NCORES):
            i = k * NCORES + c_
            if i < B:
                g = graphs[int(order[i])]
                members[c_] = int(order[i])
                if g["P3"] > 0 and g["Cp"] > 0:
                    p3m = max(p3m, g["P3"])
                    cpm = max(cpm, g["Cp"])
                    if g["start2"] is not None:
                        x2 = g["start2"] if x2 is None else min(x2, g["start2"])
        cpm = -(-cpm // ALIGN) * ALIGN
        if p3m == 0 or cpm == 0:
            continue
        if p3m > 128 and x2 is None:
            x2 = 0
        if x2 is not None:
            x2 = (x2 // ALIGN) * ALIGN
        slots.append(dict(members=members, P3=p3m, Cp=cpm, x2=x2))

    # --- stripes: (slot, n-chunk). Consecutive per slot. The second n-chunk
    # only covers the pos-column suffix [x2, Cp) it can contribute to. ---
    stripes = []
    for si, sl in enumerate(slots):
        nch = -(-sl["P3"] // 128)
        for j in range(nch):
            K = min(128, sl["P3"] - 128 * j)
            x0 = 0 if j == 0 else sl["x2"]
            stripes.append(
                dict(slot=si, chunk=j, nch=nch, K=K, W=sl["Cp"] - x0, x0=x0)
            )
    nstripes = len(stripes)

    # --- super-tile packing (ACT granularity) ---
    supertiles = []
    cur = None
    goff = 0
    for j, st in enumerate(stripes):
        if cur is None or cur["W"] + st["W"] > W_SUPER:
            cur = dict(W=0, g0=goff, stripes=[])
            supertiles.append(cur)
        st["t"] = len(supertiles) - 1
        st["soff"] = cur["W"]
        st["goff"] = goff
        cur["stripes"].append(j)
        cur["W"] += st["W"]
        goff += st["W"]
    Wtot = goff

    # --- psum col packing: 512-col banks, grouped 4 banks per psum tensor ---
    ptiles = []
    pcur = None
    for si, sl in enumerate(slots):
        if pcur is None or pcur["W"] + sl["Cp"] > PSUM_W:
            pcur = dict(W=0, slots=[])
            ptiles.append(pcur)
        sl["pt"] = len(ptiles) - 1
        sl["coff"] = pcur["W"]
        pcur["slots"].append(si)
        pcur["W"] += sl["Cp"]
    ntiles = len(ptiles)
    ngroups = -(-ntiles // PSUM_BANKS_PER_GROUP)

    # Subtracts all run on DVE: GPSIMD's tensor_scalar ucode measures
    # ~2.5us/stripe on HW (~10x the DVE cost), so it never wins.
    for sup in supertiles:
        sup["eng"] = 0
        for j in sup["stripes"]:
            stripes[j]["eng"] = 0

    # --- per-core input arrays ---
    posrow = np.zeros((NCORES, max(Wtot, 1)), dtype=np.float32)
    negcol = np.zeros((NCORES, 128, max(nstripes, 1)), dtype=np.float32)
    bmask = np.zeros((NCORES, 128, max(3 * nstripes, 1)), dtype=np.float32)
    for j, st in enumerate(stripes):
        sl = slots[st["slot"]]
        for c_ in range(NCORES):
            gi = sl["members"][c_]
            if gi is None:
                continue
            g = graphs[gi]
            if g["P3"] == 0 or g["Cp"] == 0:
                continue
            lgs = g["lgs"]
            c0 = g["c0"]
            x0 = st["x0"]
            if g["Cp"] > x0:
                posrow[c_, st["goff"] : st["goff"] + g["Cp"] - x0] = lgs[
                    c0 + x0 :
                ]
            n0 = 128 * st["chunk"]
            n1 = min(g["P3"], n0 + 128)
            if n1 > n0:
                negcol[c_, 0 : n1 - n0, j] = lgs[n0:n1]
                for s in (1, 2, 3):
                    Ps = g["P"][s - 1]
                    r1 = min(Ps, n1) - n0
                    if r1 > 0:
                        bmask[c_, 0:r1, 3 * j + (s - 1)] = 1.0

    return dict(
        graphs=graphs,
        slots=slots,
        stripes=stripes,
        supertiles=supertiles,
        ptiles=ptiles,
        nstripes=nstripes,
        ntiles=ntiles,
        ngroups=ngroups,
        Wtot=Wtot,
        n_valid=n_valid,
        posrow=posrow,
        negcol=negcol,
        bmask=bmask,
    )


def _signature(plan):
    """Program-shape signature for caching the compiled module."""
    sig = [plan["Wtot"], plan["nstripes"], plan["ntiles"], ACT_MODE]
    for st in plan["stripes"]:
        sig += [st["slot"], st["chunk"], st["K"], st["W"], st["t"], st["soff"], st["eng"]]
    for sl in plan["slots"]:
        sig += [sl["pt"], sl["coff"], sl["Cp"], sl["P3"]]
    return tuple(sig)


def _out_loc(sl):
    """(row0, col0) of a slot's [3, Cp] stripe in the dram out tensor."""
    pt = sl["pt"]
    g = pt // PSUM_BANKS_PER_GROUP
    col = (pt % PSUM_BANKS_PER_GROUP) * PSUM_W + sl["coff"]
    return 3 * g, col


def _emulate(plan):
    """Numpy emulation of the device program (for correctness of packing)."""
    GRPW = PSUM_BANKS_PER_GROUP * PSUM_W
    outs = []
    for c_ in range(NCORES):
        out = np.zeros((plan["ngroups"] * 3, GRPW), dtype=np.float32)
        for j, st in enumerate(plan["stripes"]):
            sl = plan["slots"][st["slot"]]
            w = st["W"]
            pos = plan["posrow"][c_][st["goff"] : st["goff"] + w]  # [w]
            neg = plan["negcol"][c_][0 : st["K"], j]  # [K]
            d = pos[None, :] - neg[:, None]  # [K, w] = lg_p - lg_n
            if ACT_MODE == "softplus":
                val = np.logaddexp(0.0, -d)  # softplus(-d)
            else:
                val = np.log(1.0 / (1.0 + np.exp(-d)))  # logsigmoid(d)
            bm = plan["bmask"][c_][0 : st["K"], 3 * j : 3 * j + 3]  # [K, 3]
            acc = bm.T.astype(np.float32) @ val.astype(np.float32)  # [3, w]
            r0, c0 = _out_loc(sl)
            c0 += st["x0"]
            out[r0 : r0 + 3, c0 : c0 + w] += acc
        outs.append(out)
    return outs


def _epilogue(plan, outs):
    total = 0.0
    for c_ in range(NCORES):
        out = outs[c_]
        for si, sl in enumerate(plan["slots"]):
            gi = sl["members"][c_]
            if gi is None:
                continue
            g = plan["graphs"][gi]
            if g["P3"] == 0 or g["Cp"] == 0 or not g["gvalid"]:
                continue
            r0, c0 = _out_loc(sl)
            O = out[r0 : r0 + 3, c0 : c0 + g["Cp"]].astype(np.float64)
            total += float((g["A"][:, : g["Cp"]] * O).sum())
    if ACT_MODE == "softplus":
        loss = total / plan["n_valid"]
    else:
        loss = -total / plan["n_valid"]
    return np.float32(loss)


_PROG_CACHE = {}


def _build_program(plan):
    import concourse.bass as bass  # noqa: F401
    import concourse.tile as tile
    from concourse import bacc, mybir
    from contextlib import ExitStack

    f32 = mybir.dt.float32
    nc = bacc.Bacc("TRN2", target_bir_lowering=False, debug=False,
                   num_devices=NCORES)
    GRPW = PSUM_BANKS_PER_GROUP * PSUM_W
    posrow = nc.dram_tensor("posrow", [1, max(plan["Wtot"], 1)], f32,
                            kind="ExternalInput")
    negcol = nc.dram_tensor("negcol", [128, max(plan["nstripes"], 1)], f32,
                            kind="ExternalInput")
    bmask = nc.dram_tensor("bmask", [128, max(3 * plan["nstripes"], 1)], f32,
                           kind="ExternalInput")
    out = nc.dram_tensor("out", [max(plan["ngroups"], 1) * 3, GRPW], f32,
                         kind="ExternalOutput")

    EXP = mybir.ActivationFunctionType.Exp
    LN = mybir.ActivationFunctionType.Ln

    stripes, slots = plan["stripes"], plan["slots"]
    # per psum group: index of last stripe writing it (for staging placement)
    last_stripe_of_group = {}
    for j, st in enumerate(stripes):
        grp = slots[st["slot"]]["pt"] // PSUM_BANKS_PER_GROUP
        last_stripe_of_group[grp] = j

    with tile.TileContext(nc) as tc, ExitStack() as ctx:
        cpool = ctx.enter_context(tc.tile_pool(name="const", bufs=1))
        dpool = ctx.enter_context(tc.tile_pool(name="d", bufs=3))
        lpool = ctx.enter_context(tc.tile_pool(name="ls", bufs=3))
        l2pool = ctx.enter_context(tc.tile_pool(name="ls2", bufs=3))
        ppool = ctx.enter_context(tc.tile_pool(name="ps", bufs=2, space="PSUM"))
        spool = ctx.enter_context(tc.tile_pool(name="stage", bufs=2))

        negt = cpool.tile([128, max(plan["nstripes"], 1)], f32, tag="negt")
        nc.sync.dma_start(out=negt[:, :], in_=negcol.ap()[:, :])
        bt = cpool.tile([128, max(3 * plan["nstripes"], 1)], f32, tag="bt")
        nc.sync.dma_start(out=bt[:, :], in_=bmask.ap()[:, :])
        # zero stationary + dummy row: K=1 matmuls that zero psum gap columns
        zstat = cpool.tile([1, 3], f32, tag="zstat")
        nc.vector.memset(zstat[:, :], 0.0)
        zrow = cpool.tile([1, PSUM_W], f32, tag="zrow")
        nc.vector.memset(zrow[:, :], 0.0)
        # joiners: 1-element reads that absorb DMA waits so TensorScalar
        # instructions never need more than one sync-wait slot
        joinv = cpool.tile([1, 2], f32, tag="joinv")
        nc.vector.tensor_copy(joinv[0:1, 0:1], negt[0:1, 0:1])

        def zero_gaps(pt, grp):
            # Gap columns are never read by the host; zeroing them only
            # matters for the simulator's uninitialized-psum check.
            if os.environ.get("BPR_ZERO_GAPS", "0") != "1":
                return
            ptl = plan["ptiles"]
            for b in range(PSUM_BANKS_PER_GROUP):
                pi = grp * PSUM_BANKS_PER_GROUP + b
                used = ptl[pi]["W"] if pi < len(ptl) else 0
                if used < PSUM_W:
                    gw = PSUM_W - used
                    nc.tensor.matmul(
                        out=pt[0:3, b * PSUM_W + used : (b + 1) * PSUM_W],
                        lhsT=zstat[0:1, 0:3],
                        rhs=zrow[0:1, 0:gw],
                        start=True,
                        stop=True,
                    )

        ptile_objs = {}

        for t, sup in enumerate(plan["supertiles"]):
            W = sup["W"]
            dt_ = dpool.tile([128, W], f32, tag="d")
            # broadcast pos rows across partitions via stride-0 DMA
            nsp = min(N_DMA_SPLIT, max(1, W // 256))
            bounds = [round(i * W / nsp) for i in range(nsp + 1)]
            for a, b_ in zip(bounds[:-1], bounds[1:]):
                if b_ > a:
                    src = posrow.ap()[0:1, sup["g0"] + a : sup["g0"] + b_]
                    nc.sync.dma_start(
                        out=dt_[:, a:b_], in_=src.partition_broadcast(128)
                    )
            # in-place subtract: d = lg_p - lg_n (all on DVE)
            eng = nc.vector
            for j in sup["stripes"]:
                st = stripes[j]
                K, w, so = st["K"], st["W"], st["soff"]
                eng.tensor_scalar_sub(
                    dt_[0:K, so : so + w],
                    dt_[0:K, so : so + w],
                    negt[0:K, j : j + 1],
                )
            # softplus(lg_n - lg_p) = ln(1 + exp(-(lg_p - lg_n))); Exp and Ln
            # share one ACT table set (natural_log_exp_and_others). Ln writes
            # a separate pool so exp's slot-WAR stays on the ACT engine.
            ls_ = lpool.tile([128, W], f32, tag="ls")
            nc.scalar.activation(ls_[:, :], dt_[:, :], EXP, bias=0.0, scale=-1.0)
            l2_ = l2pool.tile([128, W], f32, tag="ls2")
            nc.scalar.activation(l2_[:, :], ls_[:, :], LN, bias=1.0, scale=1.0)
            # contraction over n with prefix masks
            for j in sup["stripes"]:
                st = stripes[j]
                sl = slots[st["slot"]]
                K, w, so = st["K"], st["W"], st["soff"]
                grp = sl["pt"] // PSUM_BANKS_PER_GROUP
                if grp not in ptile_objs:
                    ptile_objs[grp] = ppool.tile(
                        [3, GRPW], f32, tag="ps", name=f"ps{grp}"
                    )
                    zero_gaps(ptile_objs[grp], grp)
                pt = ptile_objs[grp]
                c0_ = (
                    (sl["pt"] % PSUM_BANKS_PER_GROUP) * PSUM_W
                    + sl["coff"]
                    + st["x0"]
                )
                nc.tensor.matmul(
                    out=pt[0:3, c0_ : c0_ + w],
                    lhsT=bt[0:K, 3 * j : 3 * j + 3],
                    rhs=l2_[0:K, so : so + w],
                    start=(st["chunk"] == 0),
                    stop=(st["chunk"] == st["nch"] - 1),
                )
                if last_stripe_of_group.get(grp) == j:
                    stg = spool.tile([3, GRPW], f32, tag="stage",
                                     name=f"stg{grp}")
                    nc.vector.tensor_copy(joinv[0:1, 0:1], pt[0:1, 0:1])
                    nc.vector.tensor_copy(stg[:, :], pt[:, :])
                    nc.sync.dma_start(
                        out=out.ap()[3 * grp : 3 * grp + 3, :],
                        in_=stg[:, :],
                    )
                    del ptile_objs[grp]
    nc.compile()
    return nc


def _run_device(plan, trace=False):
    from concourse.bass_utils import run_bass_kernel_spmd

    sig = _signature(plan)
    if sig not in _PROG_CACHE:
        _PROG_CACHE[sig] = _build_program(plan)
    nc = _PROG_CACHE[sig]
    in_maps = [
        {
            "posrow": plan["posrow"][c_][None, :],
            "negcol": np.ascontiguousarray(plan["negcol"][c_]),
            "bmask": np.ascontiguousarray(plan["bmask"][c_]),
        }
        for c_ in range(NCORES)
    ]
    res = run_bass_kernel_spmd(
        nc, in_maps, core_ids=list(range(NCORES)), trace=trace
    )
    kernel._last_results = res
    return [res.results[c_]["out"] for c_ in range(NCORES)]


def kernel(logits, labels, s_num, _emulate_only=False, _trace=False):
    logits = np.asarray(logits)
    labels = np.asarray(labels)
    s_num = np.asarray(s_num)
    plan = _plan(logits, labels, s_num)
    if plan["nstripes"] == 0:
        return np.float32(0.0)
    if _emulate_only:
        outs = _emulate(plan)
    else:
        outs = _run_device(plan, trace=_trace)
    return _epilogue(plan, outs)


kernel._last_results = None
